# revision 49
# baseline (speedup 1.0000x reference)
"""Distributed Bass attention kernel for 8 TRN2 NeuronCores.

Problem: single-head causal attention, B=4, S=2048, d_model=1024, d_head=64.
  q = x@WQ.T+bq; k = x@WK.T+bk; v = x@WV.T+bv (v is d_model wide)
  out = softmax(causal(q@k.T)) @ v

Sharding: core = 2*b + half. Each core computes batch b, output channels
[half*512, (half+1)*512). Q/K/scores/softmax are duplicated within a batch
pair (cheap); V projection and attn@V are channel-split. No collectives.

Layout tricks:
  - x fed pre-transposed and bf16 (xT [d, S]) so projections contract
    d_model on partitions; q/k projections accumulate in fp32 PSUM with the
    bias folded in via a K=1 ones matmul.
  - scores computed transposed [keys, queries] so attn@V uses the exp'd P
    tiles directly as the stationary operand - no transposes anywhere.
  - scores matmul pads the 64-wide head dim to K=128 and uses the spare
    array rows for extra precision: lhsT = [k_hi; k_lo] (bf16 split) against
    rhs = [q_hi; q_hi] computes (k_hi + k_lo) . q_hi in one full-array pass.
  - softmax without max-subtraction (|logits| <= ~50 => exp fits fp32 fine).
    P stays unnormalized on-chip; per-query key-sums are accumulated as two
    parallel tile-sum chains (DVE + GpSimd) and exported raw - the host does
    the final 128-way partition sum and the divide in fp32 (exact).
  - attention blocks processed in reverse size order and software-pipelined:
    block j's attn@V matmuls are interleaved into block j-1's scores/exp
    emission so the in-order PE queue never stalls (keeps the HAM clock-gate
    released); dummy warm-up matmuls cover the initial DMA window.
"""

import sys

if "/opt/trn_rl_repo" not in sys.path:
    sys.path.insert(0, "/opt/trn_rl_repo")

import numpy as np

from concourse import bacc, tile, mybir
import concourse.bass as bass
from concourse.bass_utils import run_bass_kernel_spmd

B, S, D, HD = 4, 2048, 1024, 64
N_CORES = 8
CPC = 512  # output channels per core
NCHUNK = 8  # d_model / 128

f32 = mybir.dt.float32
f32r = mybir.dt.float32r
bf16 = mybir.dt.bfloat16
AF = mybir.ActivationFunctionType
ALU = mybir.AluOpType

_cache = {}


def _build():
    nc = bacc.Bacc("TRN2", target_bir_lowering=False, debug=False, num_devices=N_CORES)

    xT = nc.dram_tensor("xT", [NCHUNK, 128, S], bf16, kind="ExternalInput")
    wqkT = nc.dram_tensor("wqkT", [NCHUNK, 128, 128], bf16, kind="ExternalInput")
    bqkr = nc.dram_tensor("bqkr", [1, 128], bf16, kind="ExternalInput")
    wvT = nc.dram_tensor("wvT", [NCHUNK, 128, CPC], bf16, kind="ExternalInput")
    masks = nc.dram_tensor("masks", [4, 128, 512], bf16, kind="ExternalInput")
    out = nc.dram_tensor("out", [16, 128, CPC], f32, kind="ExternalOutput")
    rsum = nc.dram_tensor("rsum", [4, 2, 128, 512], f32, kind="ExternalOutput")

    with tile.TileContext(nc) as tc:
        with (
            tc.tile_pool(name="big", bufs=1) as big,
            tc.tile_pool(name="ppool", bufs=30) as ppool,
            tc.tile_pool(name="opool", bufs=4) as opool,
            tc.tile_pool(name="small", bufs=4) as small,
            tc.tile_pool(name="spool", bufs=4) as spool,
            tc.tile_pool(name="ps_s", bufs=4, space=bass.MemorySpace.PSUM) as ps_s,
            tc.tile_pool(name="ps_v", bufs=2, space=bass.MemorySpace.PSUM) as ps_v,
            tc.tile_pool(name="ps_o", bufs=2, space=bass.MemorySpace.PSUM) as ps_o,
        ):
            # persistent SBUF tiles
            xt = big.tile([128, NCHUNK, S], bf16, tag="xt")  # 32KB/p
            wqk = big.tile([128, NCHUNK, 128], bf16, tag="wqk")  # 2KB/p
            wv = big.tile([128, NCHUNK, CPC], bf16, tag="wv")  # 8KB/p
            bqk_sb = big.tile([1, 128], bf16, tag="bqk")
            mask_sb = big.tile([128, 4, 512], bf16, tag="mask")  # 4KB/p
            qkhi = big.tile([128, S], bf16, tag="qkhi")  # rows q_hi / k_hi
            khiklo = big.tile([128, S], bf16, tag="khiklo")  # [k_hi; k_lo]
            qhiqhi = big.tile([128, S], bf16, tag="qhiqhi")  # [q_hi; q_hi]
            v_sb = big.tile([128, 16, CPC], bf16, tag="v")  # 16KB/p
            ones_b = big.tile([128, 512], bf16, tag="ones_b")

            # input DMAs, most-urgent first: the tiny bias row unblocks the
            # first PE instruction, then wqk+xt feed the projection matmuls
            nc.sync.dma_start(out=bqk_sb[:, :], in_=bqkr[:, :])
            for c in range(NCHUNK):
                nc.sync.dma_start(out=wqk[:, c, :], in_=wqkT[c, :, :])
                nc.sync.dma_start(out=xt[:, c, :], in_=xT[c, :, :])
            for c in range(NCHUNK):
                nc.sync.dma_start(out=wv[:, c, :], in_=wvT[c, :, :])
            for m in range(4):
                nc.sync.dma_start(out=mask_sb[:, m, :], in_=masks[m, :, :])
            nc.vector.memset(ones_b[:, :], 1.0)

            # PE warmup: dummy matmuls on the ones tile while input DMA
            # streams, so the HAM clock-gate is released before real work.
            # Chained into out[0] (overwritten later) so DCE keeps them.
            warm_ps = ps_s.tile([128, 128], f32, tag="scps", name="warm_ps")
            for w in range(16):
                nc.tensor.matmul(
                    warm_ps[:, :],
                    ones_b[:, 0:128],
                    ones_b[:, 0:128],
                    start=(w == 0),
                    stop=(w == 15),
                )
            warm_sb = small.tile([128, 128], f32, tag="warm", name="warm_sb")
            nc.vector.tensor_copy(warm_sb[:, :], warm_ps[:, :])
            nc.sync.dma_start(out=out[0, :, 0:128], in_=warm_sb[:, :])

            # ---- Q/K projection: qkT [128h (64 q + 64 k), S] ----
            # chunk-outer (pairs of blocks) so compute starts on chunk 0;
            # accumulators share the scores pool's PSUM slots (same tag)
            for pair in range(2):
                qk_ps = [
                    ps_s.tile([128, 512], f32, tag="scps", name=f"qkps{pair}{jj}")
                    for jj in range(2)
                ]
                for jj in range(2):
                    nc.tensor.matmul(
                        qk_ps[jj][:, :],
                        bqk_sb[:, :],
                        ones_b[0:1, 0:512],
                        start=True,
                        stop=False,
                    )
                for c in range(NCHUNK):
                    for jj in range(2):
                        j = 2 * pair + jj
                        nc.tensor.matmul(
                            qk_ps[jj][:, :],
                            wqk[:, c, :],
                            xt[:, c, 512 * j : 512 * (j + 1)],
                            start=False,
                            stop=(c == NCHUNK - 1),
                        )
                for jj in range(2):
                    j = 2 * pair + jj
                    blk = slice(512 * j, 512 * (j + 1))
                    nc.scalar.copy(qkhi[:, blk], qk_ps[jj][:, :])
                    # k_lo = (k + bias) - k_hi, straight into khiklo rows 64+
                    nc.vector.tensor_tensor(
                        khiklo[64:128, blk],
                        qk_ps[jj][64:128, :],
                        qkhi[64:128, blk],
                        ALU.subtract,
                    )
                    nc.sync.dma_start(out=khiklo[0:64, blk], in_=qkhi[64:128, blk])
                    nc.sync.dma_start(out=qhiqhi[0:64, blk], in_=qkhi[0:64, blk])
                    nc.sync.dma_start(out=qhiqhi[64:128, blk], in_=qkhi[0:64, blk])

            # ---- attention: blocks in reverse order, software-pipelined ----
            # Section s runs block j's scores/exp/rowsum while the previous
            # section's (larger) block does its attn@V - interleaved in PE
            # program order so neither phase stalls the in-order PE queue.
            def emit_scores(j, i, Ssum):
                # K=128 single matmul: rows 0-63 k_hi x q_hi, rows 64-127
                # k_lo x q_hi => scores = (k_hi + k_lo) . q_hi
                sc_ps = ps_s.tile([128, 512], f32, tag="scps", name=f"scps{j}_{i}")
                nc.tensor.matmul(
                    sc_ps[:, :],
                    khiklo[:, 128 * i : 128 * (i + 1)],
                    qhiqhi[:, 512 * j : 512 * (j + 1)],
                    start=True,
                    stop=True,
                )
                p = ppool.tile([128, 512], bf16, tag="p", name=f"p{j}_{i}")
                nc.scalar.activation(p[:, :], sc_ps[:, :], AF.Exp)
                if i >= 4 * j:
                    nc.vector.tensor_tensor(
                        p[:, :], p[:, :], mask_sb[:, i - 4 * j, :], ALU.mult
                    )
                eng = nc.vector if i % 2 == 0 else nc.gpsimd
                Sc = Ssum[i % 2]
                if i < 2:
                    eng.tensor_copy(Sc[:, :], p[:, :])
                else:
                    eng.tensor_tensor(Sc[:, :], Sc[:, :], p[:, :], ALU.add)
                return [p]

            def attnv_ops(j, P, reverse=False):
                ops = []
                for tq in ([3, 2, 1, 0] if reverse else range(4)):
                    t = 4 * j + tq
                    ops.append(("alloc", t))
                    for i in range(t + 1):
                        ops.append(("mm", t, i))
                    ops.append(("evac", t))
                return ops

            def emit_attnv_op(op, P, state):
                if op[0] == "alloc":
                    t = op[1]
                    state[t] = ps_o.tile([128, CPC], f32, tag="ops", name=f"ops{t}")
                elif op[0] == "mm":
                    _, t, i = op
                    nc.tensor.matmul(
                        state[t][:, :],
                        P[i][:, 128 * (t % 4) : 128 * (t % 4) + 128],
                        v_sb[:, i, :],
                        start=(i == 0),
                        stop=(i == t),
                    )
                else:
                    t = op[1]
                    o_sb = opool.tile([128, CPC], f32, tag="osb", name=f"osb{t}")
                    nc.scalar.copy(o_sb[:, :], state[t][:, :])
                    nc.sync.dma_start(out=out[t, :, :], in_=o_sb[:, :])

            # ---- V projection, with block 3's scores interleaved into the
            # tail so its exp chain (ACT) finishes before attn@V needs P ----
            Ssum3 = [
                spool.tile([128, 512], f32, tag=f"S{c}", name=f"S3_{c}")
                for c in range(2)
            ]
            P3 = []
            for t in range(16):
                v_ps = ps_v.tile([128, CPC], f32, tag="vps")
                for c in range(NCHUNK):
                    nc.tensor.matmul(
                        v_ps[:, :],
                        xt[:, c, 128 * t : 128 * (t + 1)],
                        wv[:, c, :],
                        start=(c == 0),
                        stop=(c == NCHUNK - 1),
                    )
                if t % 2 == 1:
                    nc.vector.tensor_copy(v_sb[:, t, :], v_ps[:, :])
                else:
                    nc.scalar.copy(v_sb[:, t, :], v_ps[:, :])
                if 6 <= t < 14:
                    P3.extend(emit_scores(3, 2 * (t - 6), Ssum3))
                    P3.extend(emit_scores(3, 2 * (t - 6) + 1, Ssum3))
            for c in range(2):
                nc.gpsimd.dma_start(out=rsum[3, c, :, :], in_=Ssum3[c][:, :])

            prev = (3, P3)  # block 3 scored during vproj; attn@V pending
            for j in [2, 1, 0, None]:
                av = attnv_ops(*prev, reverse=(j is None)) if prev is not None else []
                avP = prev[1] if prev is not None else None
                av_state = {}
                if j is None:
                    for op in av:
                        emit_attnv_op(op, avP, av_state)
                    break
                n = 4 * j + 4
                Ssum = [
                    spool.tile([128, 512], f32, tag=f"S{c}", name=f"S{j}_{c}")
                    for c in range(2)
                ]
                P = []
                A = list(range(n))  # score emissions
                # front-load a couple of score pairs, then interleave the
                # previous block's attn@V ops
                front = min(3, len(A))
                k_av = 0
                for idx, i in enumerate(A):
                    P.extend(emit_scores(j, i, Ssum))
                    if idx >= front - 1:
                        want = (idx + 1 - front + 1) * len(av) / max(
                            1, len(A) - front + 1
                        )
                        while k_av < len(av) and k_av < want:
                            emit_attnv_op(av[k_av], avP, av_state)
                            k_av += 1
                while k_av < len(av):
                    emit_attnv_op(av[k_av], avP, av_state)
                    k_av += 1
                # gpsimd queue: these wait on the slow add-chains and must
                # not block the sync queue's out[t] DMAs behind them
                for c in range(2):
                    nc.gpsimd.dma_start(out=rsum[j, c, :, :], in_=Ssum[c][:, :])
                prev = (j, P)

    nc.compile()
    return nc


def _get_nc():
    if "nc" not in _cache:
        _cache["nc"] = _build()
    return _cache["nc"]


def _prep_in_maps(x, WQ_w, WQ_b, WK_w, WK_b, WV_w, WV_b):
    bf = mybir.dt.np(bf16)
    wqk = np.concatenate([WQ_w, WK_w], axis=0)  # [128, D]
    wqkT = np.ascontiguousarray(wqk.T.reshape(NCHUNK, 128, 128)).astype(bf)
    bqkr = np.concatenate([WQ_b, WK_b]).reshape(1, 128).astype(bf)

    # masks[m, kk, qq] = 1 if 128*m + kk <= qq else 0
    kk = np.arange(128)[:, None]
    qq = np.arange(512)[None, :]
    masks = np.stack([(128 * m + kk <= qq) for m in range(4)], axis=0).astype(bf)

    in_maps = []
    for core in range(N_CORES):
        b, half = core // 2, core % 2
        xTb = np.ascontiguousarray(x[b].T).reshape(NCHUNK, 128, S)
        wv_sl = WV_w[half * CPC : (half + 1) * CPC]  # [CPC, D]
        wvT = np.ascontiguousarray(wv_sl.T).reshape(NCHUNK, 128, CPC)
        in_maps.append(
            {
                "xT": xTb.astype(bf),
                "wqkT": wqkT,
                "bqkr": bqkr,
                "wvT": wvT.astype(bf),
                "masks": masks,
            }
        )
    return in_maps


def _run(in_maps, trace=False, **kw):
    nc = _get_nc()
    return run_bass_kernel_spmd(
        nc, in_maps, core_ids=list(range(N_CORES)), trace=trace, **kw
    )


def kernel(x, WQ_w, WQ_b, WK_w, WK_b, WV_w, WV_b):
    x = np.asarray(x, dtype=np.float32)
    in_maps = _prep_in_maps(
        x,
        np.asarray(WQ_w, np.float32),
        np.asarray(WQ_b, np.float32),
        np.asarray(WK_w, np.float32),
        np.asarray(WK_b, np.float32),
        np.asarray(WV_w, np.float32),
        np.asarray(WV_b, np.float32),
    )
    res = _run(in_maps, trace=False)
    out = np.empty((B, S, D), dtype=np.float32)
    for core in range(N_CORES):
        b, half = core // 2, core % 2
        shard = res.results[core]["out"].reshape(S, CPC)
        if half == 0:
            rs = res.results[core]["rsum"].sum(axis=(1, 2)).reshape(S)
            out[b] = 0.0
        out[b, :, half * CPC : (half + 1) * CPC] = shard
        if half == 1:
            out[b] /= rs[:, None]
    out += np.asarray(WV_b, np.float32)[None, None, :]
    return out
